# revision 8
# baseline (speedup 1.0000x reference)
"""Multi-head self-attention TRN2 kernel, 8-core head-parallel.

Problem: x[2,2048,1024], W_qkv[1024,3072], b_qkv[3072], W_out[1024,1024],
b_out[1024]; 16 heads, head_dim 64. Each core handles 2 heads.

Per-core pipeline (all-transposed layouts; fp32r matmuls, fp16 E/V):
  1. qkvT[384,4096] = Wc.T @ xT + bc      (xT transposed on host)
  2. V^T -> V natural via PE transpose, ones column appended (fused rowsum)
  3. per (b, chunk-pair, k-block, head): S^T = K_h Q_h^T (PSUM) ->
     exp(fp16, scale=1/8) -> O'^T += V'_h^T E^T (PSUM, M=65: row 64=rowsum)
  4. normalize: recip(rowsum) -> PE outer-product broadcast -> attnT
  5. outT_partial[1024,4096] = WoC.T @ attnT  -> DRAM

Host: sum partials over 8 cores, transpose, + b_out.
"""
import numpy as np

import concourse.bass as bass
import concourse.mybir as mybir
import concourse.tile as tile
from concourse import bacc
from concourse.bass_utils import run_bass_kernel_spmd

F32 = mybir.dt.float32
F32R = mybir.dt.float32r
FP16 = mybir.dt.float16

B, T, D = 2, 2048, 1024
H, HD = 16, 64
NCORES = 8
HPC = H // NCORES          # heads per core = 2
BT = B * T                 # 4096 tokens
KO = D // 128              # 8 k-tiles over embed dim
NCH = BT // 512            # 8 token-chunks of 512
FEATS = 3 * HPC * HD       # 384 qkv features per core
NKB = T // 128             # 16 k-blocks per batch
ACT_EXP = mybir.ActivationFunctionType.Exp


_QKV_TAGS = ("s0", "s1", "s2", "s3", "s4", "s5")


def build_kernel(loop=0, phases=("qkv", "vt", "attn", "out")):
    """Build the SPMD Bass program. loop>0 wraps the body in For_i (timing)."""
    nc = bacc.Bacc("TRN2", target_bir_lowering=False, debug=False,
                   num_devices=NCORES)
    xT_d = nc.dram_tensor("xT", [D, BT], F32R, kind="ExternalInput").ap()
    wc_d = nc.dram_tensor("wc", [D, FEATS], F32R, kind="ExternalInput").ap()
    bc_d = nc.dram_tensor("bc", [128, 3], F32, kind="ExternalInput").ap()
    wo_d = nc.dram_tensor("wo", [128, D], F32R, kind="ExternalInput").ap()
    tick_d = nc.dram_tensor("tick", [1, 1], F32, kind="ExternalInput").ap()
    outT_d = nc.dram_tensor("outT", [D, BT], F32, kind="ExternalOutput").ap()
    tock_d = nc.dram_tensor("tock", [1, 1], F32, kind="ExternalOutput").ap()

    with tile.TileContext(nc) as tc:
        with (
            tc.tile_pool(name="cst", bufs=1) as cst,
            tc.tile_pool(name="xin", bufs=3) as xin,
            tc.tile_pool(name="qkv", bufs=1) as qkvp,
            tc.tile_pool(name="att", bufs=1) as attp,
            tc.tile_pool(name="etile", bufs=4) as etp,
            tc.tile_pool(name="nrm", bufs=2) as nrm,
            tc.tile_pool(name="psum", bufs=1, space="PSUM") as psp,
        ):
            # ---- constants / weights ----
            ident = cst.tile([128, 128], F32, tag="ident")
            from concourse.masks import make_identity
            make_identity(nc, ident[:])
            ones_f = cst.tile([1, 64], F32, tag="ones_f")
            nc.vector.memset(ones_f[:], 1.0)
            ones_r = cst.tile([1, 64], F32R, tag="ones_r")
            nc.vector.tensor_copy(ones_r[:], ones_f[:])

            wc_sb = cst.tile([128, KO, FEATS], F32R, tag="wc")
            nc.sync.dma_start(wc_sb[:], wc_d.rearrange("(ko ki) f -> ki ko f", ki=128))
            bc_sb = cst.tile([128, 3], F32, tag="bc")
            nc.sync.dma_start(bc_sb[:], bc_d)
            wo_sb = cst.tile([128, D], F32R, tag="wo")
            nc.sync.dma_start(wo_sb[:], wo_d)

            # persistent SBUF state
            qkvT = [qkvp.tile([128, BT], F32R, tag=f"qkvT{t}", name=f"qkvT{t}")
                    for t in range(3)]               # QT, KT, VT
            vn = attp.tile([128, 2 * NKB, 2 * 65], FP16, tag="vn")  # V'|ones per k-block
            attnT = attp.tile([128, BT], F32R, tag="attnT")

            def body():
                nc.vector.memset(vn[:, :, 64:65], 1.0)
                nc.vector.memset(vn[:, :, 129:130], 1.0)
                # ---- phase 1: QKV^T projection ----
                xTr = xT_d.rearrange("(ko ki) t -> ki ko t", ki=128)
                for ch in range(NCH if "qkv" in phases else 0):
                    xts = []
                    for ko in range(KO):
                        xk = xin.tile([128, 512], F32R, tag=f"xt{ko % 2}",
                                      name=f"xt{ch}_{ko}", bufs=3)
                        nc.sync.dma_start(xk[:], xTr[:, ko, ch * 512:(ch + 1) * 512])
                        xts.append(xk)
                    ps = [psp.tile([128, 512], F32, tag=_QKV_TAGS[3 * (ch % 2) + t], name=f"psq{t}_{ch}")
                          for t in range(3)]
                    for ko in range(KO):
                        for t in range(3):
                            nc.tensor.matmul(
                                ps[t][:],
                                wc_sb[:, ko, t * 128:t * 128 + 128],
                                xts[ko][:], start=(ko == 0), stop=(ko == KO - 1))
                    for t in range(3):
                        nc.vector.tensor_scalar_add(
                            qkvT[t][:, ch * 512:(ch + 1) * 512], ps[t][:], bc_sb[:, t:t + 1])
                    # V natural for this chunk's 4 token-blocks (overlaps QKV MMs)
                    if "vt" in phases:
                        for tb in range(4 * ch, 4 * ch + 4):
                            pst = psp.tile([128, 128], F32, tag=f"o{tb % 2}", name=f"pstr{tb}")
                            nc.tensor.transpose(
                                pst[:], qkvT[2][:, tb * 128:(tb + 1) * 128].bitcast(F32), ident[:])
                            nc.vector.tensor_copy(vn[:, tb, 0:64], pst[:, 0:64])
                            nc.vector.tensor_copy(vn[:, tb, 65:129], pst[:, 64:128])



                # ---- phase 3+4: attention ----
                QT, KT = qkvT[0], qkvT[1]
                if "attn" not in phases:
                    nc.vector.memset(attnT[:].bitcast(F32), 0.0)

                def outproj(chunks):
                    if "out" not in phases:
                        return
                    for ch in chunks:
                        for ft in range(D // 128):
                            p = psp.tile([128, 512], F32, tag=f"s{ft % 6}",
                                         name=f"psoj{ch}_{ft}")
                            nc.tensor.matmul(
                                p[:], wo_sb[:, ft * 128:(ft + 1) * 128],
                                attnT[:, ch * 512:(ch + 1) * 512],
                                start=True, stop=True)
                            st = xin.tile([128, 512], F32, tag="ostage", name=f"ost{ch}_{ft}")
                            nc.vector.tensor_copy(st[:], p[:])
                            nc.sync.dma_start(
                                outT_d[ft * 128:(ft + 1) * 128, ch * 512:(ch + 1) * 512], st[:])

                for b in range(B if "attn" in phases else 0):
                    for ch in range(4):          # q chunk of 512 within batch b
                        qo = b * T + ch * 512
                        po = [psp.tile([65, 512], F32, tag=f"o{h}",
                                       name=f"po{b}{ch}{h}")
                              for h in range(2)]
                        for k in range(NKB):
                            kb = b * NKB + k
                            kcol = b * T + k * 128
                            ss, ee = [], []
                            for h in range(2):
                                s = psp.tile([128, 512], F32, tag=f"s{3 * h + k % 3}",
                                             name=f"pss{b}{ch}{h}_{k}")
                                nc.tensor.matmul(
                                    s[:],
                                    KT[h * 64:(h + 1) * 64, kcol:kcol + 128],
                                    QT[h * 64:(h + 1) * 64, qo:qo + 512],
                                    start=True, stop=True)
                                ss.append(s)
                            for h in range(2):
                                e = etp.tile([128, 512], FP16, tag=f"e{h}",
                                             name=f"e{b}{ch}{h}_{k}")
                                nc.scalar.activation(e[:], ss[h][:], ACT_EXP, scale=0.125)
                                ee.append(e)
                            for h in range(2):
                                nc.tensor.matmul(
                                    po[h][:],
                                    vn[:, kb, h * 65:(h + 1) * 65],
                                    ee[h][:],
                                    start=(k == 0), stop=(k == NKB - 1),
                                    skip_group_check=True)
                        # evacuate po quickly, then normalize from SBUF
                        for h in range(2):
                            posb = nrm.tile([65, 512], F32, tag=f"posb{h}",
                                            name=f"posb{b}{ch}{h}")
                            nc.vector.tensor_copy(posb[:], po[h][:])
                            rc = nrm.tile([1, 512], F32, tag=f"rc{h}", name=f"rc{b}{ch}{h}")
                            with nc.allow_low_precision(reason="softmax recip"):
                                nc.vector.reciprocal(rc[:], posb[64:65, :])
                            bcs = nrm.tile([64, 512], F32, tag=f"bcs{h}",
                                           name=f"bcs{b}{ch}{h}")
                            nc.gpsimd.partition_broadcast(bcs[:], rc[:])
                            nc.vector.tensor_mul(
                                attnT[h * 64:(h + 1) * 64, qo:qo + 512],
                                posb[0:64, :], bcs[:])
                        outproj([b * 4 + ch])

                if "attn" not in phases:
                    outproj(range(NCH))

            if loop > 0:
                with tc.For_i(0, loop, 1, hint_engines=(
                        mybir.EngineType.PE, mybir.EngineType.Activation,
                        mybir.EngineType.DVE, mybir.EngineType.SP)):
                    body()
            else:
                body()

            tk = cst.tile([1, 1], F32, tag="tk")
            nc.sync.dma_start(tk[:], tick_d)
            tk2 = cst.tile([1, 1], F32, tag="tk2")
            nc.scalar.copy(tk2[:], tk[:])
            nc.sync.dma_start(tock_d, tk2[:])

    nc.finalize()
    return nc


def shard_inputs(x, W_qkv, b_qkv, W_out):
    """Host-side shard prep. Returns list of per-core input dicts."""
    xT = np.ascontiguousarray(x.reshape(BT, D).T).astype(np.float32)
    in_maps = []
    tick = np.zeros((1, 1), np.float32)
    for c in range(NCORES):
        hs = [2 * c, 2 * c + 1]
        cols = []
        for blk in range(3):                       # Q, K, V column blocks
            for h in hs:
                cols.append(W_qkv[:, blk * D + h * HD: blk * D + (h + 1) * HD])
        wc = np.ascontiguousarray(np.concatenate(cols, axis=1))      # [1024, 384]
        bvals = []
        for blk in range(3):
            for h in hs:
                bvals.append(b_qkv[blk * D + h * HD: blk * D + (h + 1) * HD])
        bcol = np.concatenate(bvals)                                  # [384]
        bc = np.ascontiguousarray(bcol.reshape(3, 128).T)             # [128, 3]
        wo = np.ascontiguousarray(W_out[c * 128:(c + 1) * 128, :])    # [128, 1024]
        in_maps.append({"xT": xT, "wc": wc, "bc": bc, "wo": wo, "tick": tick})
    return in_maps


def combine_outputs(results, b_out):
    acc = results[0]["outT"].astype(np.float32).copy()
    for c in range(1, NCORES):
        acc += results[c]["outT"]
    out = acc.T + b_out[None, :]
    return out.reshape(B, T, D).astype(np.float32)


_NC_CACHE = {}


def kernel(x, W_qkv, b_qkv, W_out, b_out):
    x = np.asarray(x, dtype=np.float32)
    W_qkv = np.asarray(W_qkv, dtype=np.float32)
    b_qkv = np.asarray(b_qkv, dtype=np.float32)
    W_out = np.asarray(W_out, dtype=np.float32)
    b_out = np.asarray(b_out, dtype=np.float32)
    if "nc" not in _NC_CACHE:
        _NC_CACHE["nc"] = build_kernel()
    nc = _NC_CACHE["nc"]
    in_maps = shard_inputs(x, W_qkv, b_qkv, W_out)
    res = run_bass_kernel_spmd(nc, in_maps, core_ids=list(range(NCORES)))
    return combine_outputs(res.results, b_out)


if __name__ == "__main__":
    rng = np.random.default_rng(0)
    x = rng.standard_normal((B, T, D), dtype=np.float32)
    W_qkv = (rng.random((D, 3 * D), dtype=np.float32) - 0.5) / 16
    b_qkv = (rng.random(3 * D, dtype=np.float32) - 0.5) / 16
    W_out = (rng.random((D, D), dtype=np.float32) - 0.5) / 16
    b_out = (rng.random(D, dtype=np.float32) - 0.5) / 16
    out = kernel(x, W_qkv, b_qkv, W_out, b_out)
    print("out", out.shape, out.dtype, np.abs(out).mean())


# revision 9
# speedup vs baseline: 1.2284x; 1.2284x over previous
"""Multi-head self-attention TRN2 kernel, 8-core head-parallel.

Problem: x[2,2048,1024], W_qkv[1024,3072], b_qkv[3072], W_out[1024,1024],
b_out[1024]; 16 heads, head_dim 64. Each core handles 2 heads.

Per-core pipeline (all-transposed layouts; fp32r matmuls, fp16 E/V):
  1. qkvT[384,4096] = Wc.T @ xT + bc      (xT transposed on host)
  2. V^T -> V natural via PE transpose, ones column appended (fused rowsum)
  3. per (b, chunk-pair, k-block, head): S^T = K_h Q_h^T (PSUM) ->
     exp(fp16, scale=1/8) -> O'^T += V'_h^T E^T (PSUM, M=65: row 64=rowsum)
  4. normalize: recip(rowsum) -> PE outer-product broadcast -> attnT
  5. outT_partial[1024,4096] = WoC.T @ attnT  -> DRAM

Host: sum partials over 8 cores, transpose, + b_out.
"""
import numpy as np

import concourse.bass as bass
import concourse.mybir as mybir
import concourse.tile as tile
from concourse import bacc
from concourse.bass_utils import run_bass_kernel_spmd

F32 = mybir.dt.float32
F32R = mybir.dt.float32r
FP16 = mybir.dt.float16

B, T, D = 2, 2048, 1024
H, HD = 16, 64
NCORES = 8
HPC = H // NCORES          # heads per core = 2
BT = B * T                 # 4096 tokens
KO = D // 128              # 8 k-tiles over embed dim
NCH = BT // 512            # 8 token-chunks of 512
FEATS = 3 * HPC * HD       # 384 qkv features per core
NKB = T // 128             # 16 k-blocks per batch
ACT_EXP = mybir.ActivationFunctionType.Exp


_QKV_TAGS = ("s0", "s1", "s2", "s3", "s4", "s5")


def build_kernel(loop=0, phases=("qkv", "vt", "attn", "out")):
    """Build the SPMD Bass program. loop>0 wraps the body in For_i (timing)."""
    nc = bacc.Bacc("TRN2", target_bir_lowering=False, debug=False,
                   num_devices=NCORES)
    xT_d = nc.dram_tensor("xT", [D, BT], F32R, kind="ExternalInput").ap()
    wc_d = nc.dram_tensor("wc", [D, FEATS], F32R, kind="ExternalInput").ap()
    bc_d = nc.dram_tensor("bc", [128, 3], F32, kind="ExternalInput").ap()
    wo_d = nc.dram_tensor("wo", [128, D], F32R, kind="ExternalInput").ap()
    tick_d = nc.dram_tensor("tick", [1, 1], F32, kind="ExternalInput").ap()
    outT_d = nc.dram_tensor("outT", [D, BT], F32, kind="ExternalOutput").ap()
    tock_d = nc.dram_tensor("tock", [1, 1], F32, kind="ExternalOutput").ap()

    with tile.TileContext(nc) as tc:
        with (
            tc.tile_pool(name="cst", bufs=1) as cst,
            tc.tile_pool(name="xin", bufs=3) as xin,
            tc.tile_pool(name="qkv", bufs=1) as qkvp,
            tc.tile_pool(name="att", bufs=1) as attp,
            tc.tile_pool(name="etile", bufs=4) as etp,
            tc.tile_pool(name="nrm", bufs=2) as nrm,
            tc.tile_pool(name="psum", bufs=1, space="PSUM") as psp,
        ):
            # ---- constants / weights ----
            ident = cst.tile([128, 128], F32, tag="ident")
            from concourse.masks import make_identity
            make_identity(nc, ident[:])
            ones_f = cst.tile([1, 64], F32, tag="ones_f")
            nc.vector.memset(ones_f[:], 1.0)
            ones_r = cst.tile([1, 64], F32R, tag="ones_r")
            nc.vector.tensor_copy(ones_r[:], ones_f[:])

            wc_sb = cst.tile([128, KO, FEATS], F32R, tag="wc")
            nc.sync.dma_start(wc_sb[:], wc_d.rearrange("(ko ki) f -> ki ko f", ki=128))
            bc_sb = cst.tile([128, 3], F32, tag="bc")
            nc.sync.dma_start(bc_sb[:], bc_d)
            wo_sb = cst.tile([128, D], F32R, tag="wo")
            nc.sync.dma_start(wo_sb[:], wo_d)

            # persistent SBUF state
            qkvT = [qkvp.tile([128, BT], F32R, tag=f"qkvT{t}", name=f"qkvT{t}")
                    for t in range(3)]               # QT, KT, VT
            vn = attp.tile([128, 2 * NKB, 2 * 65], FP16, tag="vn")  # V'|ones per k-block
            attnT = attp.tile([128, BT], F32R, tag="attnT")

            def body():
                nc.vector.memset(vn[:, :, 64:65], 1.0)
                nc.vector.memset(vn[:, :, 129:130], 1.0)
                # ---- phase 1: QKV^T projection ----
                xTr = xT_d.rearrange("(ko ki) t -> ki ko t", ki=128)
                for ch in range(NCH if "qkv" in phases else 0):
                    xts = []
                    for ko in range(KO):
                        xk = xin.tile([128, 512], F32R, tag=f"xt{ko % 2}",
                                      name=f"xt{ch}_{ko}", bufs=3)
                        nc.sync.dma_start(xk[:], xTr[:, ko, ch * 512:(ch + 1) * 512])
                        xts.append(xk)
                    ps = [psp.tile([128, 512], F32, tag=_QKV_TAGS[3 * (ch % 2) + t], name=f"psq{t}_{ch}")
                          for t in range(3)]
                    for ko in range(KO):
                        for t in range(3):
                            nc.tensor.matmul(
                                ps[t][:],
                                wc_sb[:, ko, t * 128:t * 128 + 128],
                                xts[ko][:], start=(ko == 0), stop=(ko == KO - 1))
                    for t in range(3):
                        nc.vector.tensor_scalar_add(
                            qkvT[t][:, ch * 512:(ch + 1) * 512], ps[t][:], bc_sb[:, t:t + 1])
                    # V natural for this chunk's 4 token-blocks (overlaps QKV MMs)
                    if "vt" in phases:
                        for tb in range(4 * ch, 4 * ch + 4):
                            pst = psp.tile([128, 128], F32, tag=f"o{tb % 2}", name=f"pstr{tb}")
                            nc.tensor.transpose(
                                pst[:], qkvT[2][:, tb * 128:(tb + 1) * 128].bitcast(F32), ident[:])
                            nc.vector.tensor_copy(vn[:, tb, 0:64], pst[:, 0:64])
                            nc.vector.tensor_copy(vn[:, tb, 65:129], pst[:, 64:128])



                # ---- phase 3+4: attention ----
                QT, KT = qkvT[0], qkvT[1]
                if "attn" not in phases:
                    nc.vector.memset(attnT[:].bitcast(F32), 0.0)

                def outproj(chunks):
                    if "out" not in phases:
                        return
                    for ch in chunks:
                        for ft in range(D // 128):
                            p = psp.tile([128, 512], F32, tag=f"s{ft % 6}",
                                         name=f"psoj{ch}_{ft}")
                            nc.tensor.matmul(
                                p[:], wo_sb[:, ft * 128:(ft + 1) * 128],
                                attnT[:, ch * 512:(ch + 1) * 512],
                                start=True, stop=True)
                            st = xin.tile([128, 512], F32, tag="ostage", name=f"ost{ch}_{ft}")
                            nc.vector.tensor_copy(st[:], p[:])
                            nc.sync.dma_start(
                                outT_d[ft * 128:(ft + 1) * 128, ch * 512:(ch + 1) * 512], st[:])

                for b in range(B if "attn" in phases else 0):
                    for ch in range(4):          # q chunk of 512 within batch b
                        qo = b * T + ch * 512
                        po = [psp.tile([65, 512], F32, tag=f"o{h}",
                                       name=f"po{b}{ch}{h}")
                              for h in range(2)]
                        for k in range(NKB):
                            kb = b * NKB + k
                            kcol = b * T + k * 128
                            ss, ee = [], []
                            for h in range(2):
                                s = psp.tile([128, 512], F32, tag=f"s{3 * h + k % 3}",
                                             name=f"pss{b}{ch}{h}_{k}")
                                nc.tensor.matmul(
                                    s[:],
                                    KT[h * 64:(h + 1) * 64, kcol:kcol + 128],
                                    QT[h * 64:(h + 1) * 64, qo:qo + 512],
                                    start=True, stop=True)
                                ss.append(s)
                            for h in range(2):
                                e = etp.tile([128, 512], FP16, tag=f"e{h}",
                                             name=f"e{b}{ch}{h}_{k}")
                                nc.scalar.activation(e[:], ss[h][:], ACT_EXP, scale=0.125)
                                ee.append(e)
                            for h in range(2):
                                nc.tensor.matmul(
                                    po[h][:],
                                    vn[:, kb, h * 65:(h + 1) * 65],
                                    ee[h][:],
                                    start=(k == 0), stop=(k == NKB - 1),
                                    skip_group_check=True)
                        # evacuate po quickly, then normalize from SBUF
                        for h in range(2):
                            posb = nrm.tile([65, 512], F32, tag=f"posb{h}",
                                            name=f"posb{b}{ch}{h}")
                            nc.vector.tensor_copy(posb[:], po[h][:])
                            rc = nrm.tile([1, 512], F32, tag=f"rc{h}", name=f"rc{b}{ch}{h}")
                            with nc.allow_low_precision(reason="softmax recip"):
                                nc.vector.reciprocal(rc[:], posb[64:65, :])
                            bcs = nrm.tile([64, 512], F32, tag=f"bcs{h}",
                                           name=f"bcs{b}{ch}{h}")
                            nc.gpsimd.partition_broadcast(bcs[:], rc[:])
                            nc.vector.tensor_mul(
                                attnT[h * 64:(h + 1) * 64, qo:qo + 512],
                                posb[0:64, :], bcs[:])
                    outproj(range(b * 4, b * 4 + 4))

                if "attn" not in phases:
                    outproj(range(NCH))

            if loop > 0:
                with tc.For_i(0, loop, 1, hint_engines=(
                        mybir.EngineType.PE, mybir.EngineType.Activation,
                        mybir.EngineType.DVE, mybir.EngineType.SP)):
                    body()
            else:
                body()

            tk = cst.tile([1, 1], F32, tag="tk")
            nc.sync.dma_start(tk[:], tick_d)
            tk2 = cst.tile([1, 1], F32, tag="tk2")
            nc.scalar.copy(tk2[:], tk[:])
            nc.sync.dma_start(tock_d, tk2[:])

    nc.finalize()
    return nc


def shard_inputs(x, W_qkv, b_qkv, W_out):
    """Host-side shard prep. Returns list of per-core input dicts."""
    xT = np.ascontiguousarray(x.reshape(BT, D).T).astype(np.float32)
    in_maps = []
    tick = np.zeros((1, 1), np.float32)
    for c in range(NCORES):
        hs = [2 * c, 2 * c + 1]
        cols = []
        for blk in range(3):                       # Q, K, V column blocks
            for h in hs:
                cols.append(W_qkv[:, blk * D + h * HD: blk * D + (h + 1) * HD])
        wc = np.ascontiguousarray(np.concatenate(cols, axis=1))      # [1024, 384]
        bvals = []
        for blk in range(3):
            for h in hs:
                bvals.append(b_qkv[blk * D + h * HD: blk * D + (h + 1) * HD])
        bcol = np.concatenate(bvals)                                  # [384]
        bc = np.ascontiguousarray(bcol.reshape(3, 128).T)             # [128, 3]
        wo = np.ascontiguousarray(W_out[c * 128:(c + 1) * 128, :])    # [128, 1024]
        in_maps.append({"xT": xT, "wc": wc, "bc": bc, "wo": wo, "tick": tick})
    return in_maps


def combine_outputs(results, b_out):
    acc = results[0]["outT"].astype(np.float32).copy()
    for c in range(1, NCORES):
        acc += results[c]["outT"]
    out = acc.T + b_out[None, :]
    return out.reshape(B, T, D).astype(np.float32)


_NC_CACHE = {}


def kernel(x, W_qkv, b_qkv, W_out, b_out):
    x = np.asarray(x, dtype=np.float32)
    W_qkv = np.asarray(W_qkv, dtype=np.float32)
    b_qkv = np.asarray(b_qkv, dtype=np.float32)
    W_out = np.asarray(W_out, dtype=np.float32)
    b_out = np.asarray(b_out, dtype=np.float32)
    if "nc" not in _NC_CACHE:
        _NC_CACHE["nc"] = build_kernel()
    nc = _NC_CACHE["nc"]
    in_maps = shard_inputs(x, W_qkv, b_qkv, W_out)
    res = run_bass_kernel_spmd(nc, in_maps, core_ids=list(range(NCORES)))
    return combine_outputs(res.results, b_out)


if __name__ == "__main__":
    rng = np.random.default_rng(0)
    x = rng.standard_normal((B, T, D), dtype=np.float32)
    W_qkv = (rng.random((D, 3 * D), dtype=np.float32) - 0.5) / 16
    b_qkv = (rng.random(3 * D, dtype=np.float32) - 0.5) / 16
    W_out = (rng.random((D, D), dtype=np.float32) - 0.5) / 16
    b_out = (rng.random(D, dtype=np.float32) - 0.5) / 16
    out = kernel(x, W_qkv, b_qkv, W_out, b_out)
    print("out", out.shape, out.dtype, np.abs(out).mean())
